# revision 1
# baseline (speedup 1.0000x reference)
"""LocallyConnected2d kernel for 8 TRN2 NeuronCores (Bass/Tile).

Problem (hardcoded):
  features [32, 64, 64, 64] f32, weights [62, 62, 64, 64, 3, 3] f32,
  bias [62, 62, 64] f32 -> out [32, 64, 62, 62] f32
  out[b,o,h,w] = sum_{c,i,j} x[b,c,h+i,w+j] * W[h,w,o,c,i,j] + bias[h,w,o]

Strategy:
  - Shard over Hout: 8 cores x 8 output rows (bands [0,8,...,48,54], the last
    two overlap; host takes canonical rows from each core).
  - bf16 on the PE, fp32 PSUM accumulate. Contraction (c,i,j)=576 per output
    location via 14 matmuls per location-group, built on a host-baked
    "dual shifted" feature layout (partition p<64: x[c,t,w]; p>=64 carries a
    shifted copy) so a [128,32] AP slice is a ready im2col patch
    (batch = stationary cols).
  - Work unit = (half-band hg, group of 4 w): PSUM tile [128,256] with
    partitions=(4w x 32b) via col tile_position and free=(4 output rows x 64
    cout). ONE accumulation group per tile (single start=True; per-element
    has_written gives overwrite-on-first-touch) -> no mid-tile start stalls.
  - Matmuls grouped by stationary: a patch at absolute row t serves all
    (out-row j, kernel-row r) with j+r=t-hl in ONE matmul with a wide moving
    operand (weights host-concatenated, N up to 192).
  - DMA spread over both HWDGE rings (sync/scalar) + SWDGE (gpsimd).
  - Host: shard/pack inputs, unpack outS dumps, add bias, assemble f32 out.
"""

import numpy as np
import ml_dtypes

BF16 = ml_dtypes.bfloat16

B, CIN, COUT = 32, 64, 64
H = W = 64
HOUT = WOUT = 62
NCORES = 8
STARTS = [0, 8, 16, 24, 32, 40, 48, 54]

# t-group geometry: tau = t - hl in 0..5; valid out-rows j in [jlo, jhi]
TAUS = list(range(6))
JLO = [max(0, t - 2) for t in TAUS]
JHI = [min(3, t) for t in TAUS]
NV = [hi - lo + 1 for lo, hi in zip(JLO, JHI)]          # [1,2,3,3,2,1]
TBASE = [0]
for t in TAUS:
    TBASE.append(TBASE[-1] + 4 * NV[t] * 64)            # per-(tau) base col
WR_COLS = TBASE[-1]                                      # 3072

_STATE = {}


def _build_program():
    import concourse.tile as tile
    from concourse import bacc, mybir

    bf = mybir.dt.bfloat16
    f32 = mybir.dt.float32

    nc = bacc.Bacc(None, target_bir_lowering=False)
    featA = nc.dram_tensor("featA", [128, 10, 64, 32], bf, kind="ExternalInput")
    featB = nc.dram_tensor("featB", [128, 10, 64, 32], bf, kind="ExternalInput")
    # wr||w3||w4(padded to 128p) merged: the whole per-group weight stream is
    # ONE sequential 1.25MB transfer on a single ring (ring mixing at the
    # SDMA engines costs ~20% HBM efficiency)
    wk_d = nc.dram_tensor("wk", [2, 16, 128, WR_COLS + 2048], bf,
                          kind="ExternalInput")
    outS = nc.dram_tensor("outS", [2, 128, 4096], bf, kind="ExternalOutput")

    with tile.TileContext(nc) as tc:
        with tc.tile_pool(name="feat", bufs=1) as fpool, \
             tc.tile_pool(name="wk", bufs=6) as wkpool, \
             tc.tile_pool(name="st", bufs=2) as spool, \
             tc.tile_pool(name="ps", bufs=8, space="PSUM") as pspool:
            # featA on the scalar ring (sync is reserved for the weight
            # stream), row-chunked so early matmuls unblock sooner; featB is
            # derived on-chip from featA via SBUF->SBUF DMA (h+1 shift),
            # range-split to follow the featA chunks.
            fA = fpool.tile([128, 10, 64, 32], bf)
            nc.scalar.dma_start(fA[:, 0:6], featA[:, 0:6])
            nc.scalar.dma_start(fA[:, 6:10], featA[:, 6:10])
            fB = fpool.tile([128, 10, 64, 32], bf)
            nc.gpsimd.dma_start(fB[:, 0:6], featB[:, 0:6])
            nc.gpsimd.dma_start(fB[:, 6:10], featB[:, 6:10])
            # zero operands for the psum-clearing matmul (see below)
            zl = fpool.tile([1, 128], bf)
            nc.gpsimd.memset(zl[:], 0.0)
            zr = fpool.tile([1, 256], bf)
            nc.gpsimd.memset(zr[:], 0.0)
            for hg in range(2):
                hl = 4 * hg
                S = spool.tile([128, 4096], bf)
                for wg in range(16):
                    w0 = min(4 * wg, 58)   # last group overlaps: w 58..61
                    wk = wkpool.tile([128, WR_COLS + 2048], bf)
                    nc.sync.dma_start(wk[:], wk_d[hg, wg])
                    wr = wk[:, 0:WR_COLS]
                    w3 = wk[:, WR_COLS:WR_COLS + 1024]
                    w4 = wk[0:64, WR_COLS + 1024:WR_COLS + 2048]

                    ps = pspool.tile([128, 256], f32)
                    # K=1 zeroing matmul over the WHOLE tile: starts the
                    # accumulation group, zeroes every element, and (because
                    # its output overlaps all later MMs) forces the scheduler
                    # to keep it first; all real MMs are then pure order-free
                    # flags=0 accumulates.
                    nc.tensor.matmul(ps[:, :], zl[:], zr[:],
                                     start=True, stop=False,
                                     tile_position=(0, 0))
                    for tau in TAUS:
                        nv, jlo = NV[tau], JLO[tau]
                        for g in range(4):
                            off = TBASE[tau] + g * nv * 64
                            nc.tensor.matmul(
                                ps[32 * g:32 * g + 32,
                                   64 * jlo:64 * (jlo + nv)],
                                fA[:, hl + tau, w0 + g, :],
                                wr[:, off:off + nv * 64],
                                start=False, stop=False,
                                tile_position=(0, 32 * g),
                            )
                    for j in range(4):
                        for g in range(4):
                            off = (j * 4 + g) * 64
                            nc.tensor.matmul(
                                ps[32 * g:32 * g + 32, 64 * j:64 * j + 64],
                                fA[0:64, hl + j + 2, w0 + g + 2, :],
                                w4[:, off:off + 64],
                                start=False, stop=False,
                                tile_position=(0, 32 * g),
                            )
                    # fB-dependent matmuls last (startup slack for featB)
                    for j in range(4):
                        for g in range(4):
                            off = (j * 4 + g) * 64
                            nc.tensor.matmul(
                                ps[32 * g:32 * g + 32, 64 * j:64 * j + 64],
                                fB[:, hl + j, w0 + g + 2, :],
                                w3[:, off:off + 64],
                                start=False, stop=(j == 3 and g == 3),
                                tile_position=(0, 32 * g),
                            )
                    nc.vector.tensor_copy(S[:, 256 * wg:256 * wg + 256],
                                          ps[:])
                nc.scalar.dma_start(outS[hg], S[:])
    nc.compile()
    return nc


def _get_nc():
    if "nc" not in _STATE:
        _STATE["nc"] = _build_program()
    return _STATE["nc"]


def _prep_inputs(features, weights):
    """Build the 8 per-core input dicts (bf16, device layouts)."""
    x = np.asarray(features, dtype=np.float32)
    Wt = np.asarray(weights, dtype=np.float32)

    # w-slot -> real w: last group overlaps (w 58..61), no padding needed
    widx = list(range(60)) + [58, 59, 60, 61]

    in_maps = []
    for s in STARTS:
        xt = x[:, :, s:s + 10, :].transpose(1, 2, 3, 0)  # [c, 10, 64, b]
        fA = np.zeros((128, 10, 64, 32), dtype=BF16)
        fA[:64] = xt
        fA[64:, :, :63, :] = xt[:, :, 1:, :]             # w+1 shift
        fB = np.zeros((128, 10, 64, 32), dtype=BF16)
        fB[:64] = xt
        fB[64:, :9] = xt[:, 1:, :, :]                    # h+1 shift

        Wb = Wt[s:s + 8]                                  # [8, 62, o, c, 3, 3]
        Wsel = Wb[:, widx]                                # [8, 64slots, o, c, 3, 3]
        WT = Wsel.transpose(4, 5, 3, 0, 1, 2)             # [i, jw, c, 8h, 64w, o]

        # wr: t-grouped ktiles (cells (r,0)|(r,1)); cols per (tau,g):
        #   q=0..nv-1 -> j=jlo+q, r=tau-j; value(d,c,o)=W[h,w,o,c,r,d]
        wr = np.zeros((2, 16, 128, WR_COLS), dtype=BF16)
        for tau in TAUS:
            nv, jlo = NV[tau], JLO[tau]
            view = wr[:, :, :, TBASE[tau]:TBASE[tau + 1]].reshape(
                2, 16, 128, 4, nv, 64)
            for q in range(nv):
                j = jlo + q
                r = tau - j
                for d in range(2):
                    src = WT[r, d].reshape(CIN, 2, 4, 16, 4, COUT)[:, :, j]
                    view[:, :, d * 64:(d + 1) * 64, :, q, :] = \
                        src.transpose(1, 2, 0, 3, 4)      # [hg, wg, c, g, o]
        # w3: cells (0,2) d=0 / (1,2) d=1 ; free=(j,g,o)
        w3 = np.zeros((2, 16, 128, 1024), dtype=BF16)
        for d in range(2):
            src = WT[d, 2].reshape(CIN, 2, 4, 16, 4, COUT)
            w3[:, :, d * 64:(d + 1) * 64, :] = src.transpose(
                1, 3, 0, 2, 4, 5).reshape(2, 16, 64, 1024)
        # w4: cell (2,2)
        src = WT[2, 2].reshape(CIN, 2, 4, 16, 4, COUT)
        w4 = np.ascontiguousarray(
            src.transpose(1, 3, 0, 2, 4, 5), dtype=BF16).reshape(2, 16, 64, 1024)

        w4pad = np.zeros((2, 16, 128, 1024), dtype=BF16)
        w4pad[:, :, 0:64, :] = w4
        wk = np.concatenate([wr, w3, w4pad], axis=-1)     # [2,16,128,5120]
        in_maps.append({"featA": fA, "featB": fB, "wk": wk})
    return in_maps


def _gather(results, bias):
    out = np.zeros((B, COUT, HOUT, WOUT), dtype=np.float32)
    for core, s in enumerate(STARTS):
        arr = np.asarray(results[core]["outS"]).astype(np.float32)
        # [hg, g, b, wg, j, o] -> [b, o, hg, j, wg, g]
        arr = arr.reshape(2, 4, 32, 16, 4, 64).transpose(2, 5, 0, 4, 3, 1)
        arr = arr.reshape(32, 64, 8, 64)
        out[:, :, s:s + 8, 0:60] = arr[:, :, :, 0:60]
        out[:, :, s:s + 8, 60:62] = arr[:, :, :, 62:64]
    out += np.asarray(bias, dtype=np.float32).transpose(2, 0, 1)[None]
    return out


def _run(in_maps, trace=False, trace_cores=None):
    from concourse.bass_utils import run_bass_kernel_spmd
    nc = _get_nc()
    return run_bass_kernel_spmd(
        nc, in_maps, core_ids=list(range(NCORES)),
        trace=trace, trace_cores=trace_cores,
    )


def kernel(features, weights, bias):
    in_maps = _prep_inputs(features, weights)
    res = _run(in_maps)
    return _gather(res.results, bias)



# revision 3
# speedup vs baseline: 1.6964x; 1.6964x over previous
"""LocallyConnected2d kernel for 8 TRN2 NeuronCores (Bass/Tile).

Problem (hardcoded):
  features [32, 64, 64, 64] f32, weights [62, 62, 64, 64, 3, 3] f32,
  bias [62, 62, 64] f32 -> out [32, 64, 62, 62] f32
  out[b,o,h,w] = sum_{c,i,j} x[b,c,h+i,w+j] * W[h,w,o,c,i,j] + bias[h,w,o]

Strategy (v3 - fp8 weight stream, no featB, packed K=64 weights):
  - Shard over Hout: 8 cores x 8 output rows (bands [0,8,...,48,54], last two
    overlap; host takes canonical rows from each core).
  - Weights stream as fp8 e3m4 (x2 scale, /2 on host) = 1 B/el -> 18.9 MB/core
    with zero padding waste; activations stay bf16 (mixed-dtype matmul: only
    fp32 operands must be paired). PSUM accumulates fp32. rel err ~0.015.
  - fA layout [128=(c | c shifted w+1), w, t, b]; a [128,32] slice at (w,t) is
    an im2col patch: lower half = x(w), upper = x(w+1).
  - Per (hg=4-row half-band, wg=4-w group), outputs live in PSUM [128,256]:
    partitions=(4w x 32b) via col tile_position, free=(4j x 64 cout).
    * wr MMs (taps i in {0,1}): stationary fA[:, w0+g, hl+tau] K=128, moving
      N=nv*64, tau=j+r grouped. 24 MMs.
    * tap i=2 is K=64, tau-grouped, split across TWO psum groups because one
      accumulation group must not mix PE row-tiles (HW limitation, probed):
      - taus {0,1,4,5} (1536 cols): stationary fA[0:64, w0+g+2, hl+tau]
        (lower half), moving wk64[0:64, .], row tile 0, accumulate into main.
      - taus {2,3} (1536 cols): stationary fA[64:128, w0+g+1, hl+tau] (the
        w+1-shifted half gives the same x column), moving wk64[64:128, .],
        row tile 64, own psum tile psB with a row-64 K=1 zeroing matmul as
        group start. tau2 covers j0-2, tau3 j1-3 -> full coverage.
      Both partition halves of wk64 carry real data -> no padding bytes.
    * combine on DVE: copy psB -> S slice (bf16), tensor_add(S, ps, S).
  - DMA: fA w-chunk [0:8] FIRST on sync (655 KB -> compute starts early), the
    16 wk pair-transfers ([128,9216] fp8 = 1.18 MB) follow on sync; remaining
    fA chunks + outS dumps ride the scalar ring.
"""

import numpy as np
import ml_dtypes

BF16 = ml_dtypes.bfloat16
F8E3 = ml_dtypes.float8_e3m4
WSCALE = np.float32(2.0)

B, CIN, COUT = 32, 64, 64
H = W = 64
HOUT = WOUT = 62
NCORES = 8
STARTS = [0, 8, 16, 24, 32, 40, 48, 54]

# tau-group geometry: tau = t - hl in 0..5; valid out-rows j in [jlo, jhi]
TAUS = list(range(6))
JLO = [max(0, t - 2) for t in TAUS]
JHI = [min(3, t) for t in TAUS]
NV = [hi - lo + 1 for lo, hi in zip(JLO, JHI)]            # [1,2,3,3,2,1]
TBASE = [0]
for t in TAUS:
    TBASE.append(TBASE[-1] + 4 * NV[t] * 64)              # wr per-tau base col
WR_COLS = TBASE[-1]                                        # 3072
# K64 split: taus 0,1,4,5 -> lower half (row tile 0, main psum);
#            taus 2,3     -> upper half (row tile 64, psB)
K64_LOW_TAUS = [0, 1, 4, 5]
K64_HIGH_TAUS = [2, 3]
K64LO = {}
off = 0
for t in K64_LOW_TAUS:
    K64LO[t] = off
    off += 4 * NV[t] * 64
K64HI = {}
off = 0
for t in K64_HIGH_TAUS:
    K64HI[t] = off
    off += 4 * NV[t] * 64
K64_COLS = 1536                                            # both halves
WG_COLS = WR_COLS + K64_COLS                               # 4608

_STATE = {}


def _build_program():
    import concourse.tile as tile
    from concourse import bacc, mybir

    bf = mybir.dt.bfloat16
    f8 = mybir.dt.float8e3
    f32 = mybir.dt.float32

    nc = bacc.Bacc(None, target_bir_lowering=False)
    featA = nc.dram_tensor("featA", [128, 64, 10, 32], bf, kind="ExternalInput")
    wk_d = nc.dram_tensor("wk", [16, 128, 2 * WG_COLS], f8, kind="ExternalInput")
    outS = nc.dram_tensor("outS", [2, 128, 4096], bf, kind="ExternalOutput")

    with tile.TileContext(nc) as tc:
        with tc.tile_pool(name="feat", bufs=1) as fpool, \
             tc.tile_pool(name="wk", bufs=3) as wkpool, \
             tc.tile_pool(name="st", bufs=2) as spool, \
             tc.tile_pool(name="ps", bufs=4, space="PSUM") as pspool, \
             tc.tile_pool(name="psb", bufs=4, space="PSUM") as psbpool:
            fA = fpool.tile([128, 64, 10, 32], bf)
            # first w-chunk ahead of the weight stream on sync; the rest on
            # scalar so they don't delay wk[0]
            nc.sync.dma_start(fA[:, 0:8], featA[:, 0:8])
            nc.scalar.dma_start(fA[:, 8:24], featA[:, 8:24])
            nc.scalar.dma_start(fA[:, 24:44], featA[:, 24:44])
            nc.scalar.dma_start(fA[:, 44:64], featA[:, 44:64])
            # zeros spanning all partitions: K=1 stationary rows for the
            # psum-clearing matmuls (row 0 for main, row 64 for psB)
            zb = fpool.tile([128, 256], bf)
            nc.gpsimd.memset(zb[:], 0.0)
            for hg in range(2):
                hl = 4 * hg
                S = spool.tile([128, 4096], bf)
                for pi in range(8):
                    wk = wkpool.tile([128, 2 * WG_COLS], f8)
                    nc.sync.dma_start(wk[:], wk_d[hg * 8 + pi])
                    for sub in range(2):
                        wg = 2 * pi + sub
                        w0 = min(4 * wg, 58)  # last group overlaps: w 58..61
                        wr = wk[:, sub * WG_COLS:sub * WG_COLS + WR_COLS]
                        wk64 = wk[:, sub * WG_COLS + WR_COLS:
                                  (sub + 1) * WG_COLS]

                        ps = pspool.tile([128, 256], f32)
                        psB = psbpool.tile([128, 256], f32)
                        # K=1 zeroing matmuls start each accumulation group,
                        # zero every element, and overlap all later MMs so
                        # the scheduler keeps them first.
                        nc.tensor.matmul(ps[:, :], zb[0:1, 0:128],
                                         zb[0:1, 0:256],
                                         start=True, stop=False,
                                         tile_position=(0, 0))
                        nc.tensor.matmul(psB[:, :], zb[64:65, 0:128],
                                         zb[64:65, 0:256],
                                         start=True, stop=False,
                                         tile_position=(64, 0))
                        # taps i in {0,1}: K=128 dual-w stationaries
                        for tau in TAUS:
                            nv, jlo = NV[tau], JLO[tau]
                            for g in range(4):
                                off = TBASE[tau] + g * nv * 64
                                nc.tensor.matmul(
                                    ps[32 * g:32 * g + 32,
                                       64 * jlo:64 * (jlo + nv)],
                                    fA[:, w0 + g, hl + tau, :],
                                    wr[:, off:off + nv * 64],
                                    start=False, stop=False,
                                    tile_position=(0, 32 * g),
                                )
                        # tap i=2, taus {0,1,4,5}: K=64 lower halves -> main
                        for ti, tau in enumerate(K64_LOW_TAUS):
                            nv, jlo = NV[tau], JLO[tau]
                            for g in range(4):
                                off = K64LO[tau] + g * nv * 64
                                nc.tensor.matmul(
                                    ps[32 * g:32 * g + 32,
                                       64 * jlo:64 * (jlo + nv)],
                                    fA[0:64, w0 + g + 2, hl + tau, :],
                                    wk64[0:64, off:off + nv * 64],
                                    start=False,
                                    stop=(ti == 3 and g == 3),
                                    tile_position=(0, 32 * g),
                                )
                        # tap i=2, taus {2,3}: K=64 upper halves -> psB
                        for ti, tau in enumerate(K64_HIGH_TAUS):
                            nv, jlo = NV[tau], JLO[tau]
                            for g in range(4):
                                off = K64HI[tau] + g * nv * 64
                                nc.tensor.matmul(
                                    psB[32 * g:32 * g + 32,
                                        64 * jlo:64 * (jlo + nv)],
                                    fA[64:128, w0 + g + 1, hl + tau, :],
                                    wk64[64:128, off:off + nv * 64],
                                    start=False,
                                    stop=(ti == 1 and g == 3),
                                    tile_position=(64, 32 * g),
                                )
                        sl = S[:, 256 * wg:256 * wg + 256]
                        nc.vector.tensor_copy(sl, psB[:])
                        nc.vector.tensor_add(sl, ps[:], sl)
                nc.scalar.dma_start(outS[hg], S[:])
    nc.compile()
    return nc


def _get_nc():
    if "nc" not in _STATE:
        _STATE["nc"] = _build_program()
    return _STATE["nc"]


def _quant_w(a):
    return np.clip(a * WSCALE, -15.0, 15.0).astype(F8E3)


def _prep_inputs(features, weights):
    """Build the 8 per-core input dicts (device layouts)."""
    x = np.asarray(features, dtype=np.float32)
    Wt = np.asarray(weights, dtype=np.float32)

    # w-slot -> real w: last group overlaps (w 58..61)
    widx = list(range(60)) + [58, 59, 60, 61]

    in_maps = []
    for s in STARTS:
        xt = x[:, :, s:s + 10, :].transpose(1, 3, 2, 0)    # [c, w, t, b]
        fA = np.zeros((128, 64, 10, 32), dtype=BF16)
        fA[:64] = xt
        fA[64:, 0:63] = xt[:, 1:]                          # w+1 shift

        Wb = Wt[s:s + 8]                                   # [8, 62, o, c, 3, 3]
        Wsel = Wb[:, widx]                                 # [8, 64slots, o, c, 3, 3]
        WT = Wsel.transpose(4, 5, 3, 0, 1, 2)              # [r, i, c, 8h, 64w, o]

        wkf = np.zeros((2, 16, 128, WG_COLS), dtype=np.float32)
        # wr: taps (r, i=d); cols per (tau, g): q -> j=jlo+q, r=tau-j
        wr = wkf[:, :, :, 0:WR_COLS]
        for tau in TAUS:
            nv, jlo = NV[tau], JLO[tau]
            view = wr[:, :, :, TBASE[tau]:TBASE[tau + 1]].reshape(
                2, 16, 128, 4, nv, 64)
            for q in range(nv):
                j = jlo + q
                r = tau - j
                for d in range(2):
                    src = WT[r, d].reshape(CIN, 2, 4, 16, 4, COUT)[:, :, j]
                    view[:, :, d * 64:(d + 1) * 64, :, q, :] = \
                        src.transpose(1, 2, 0, 3, 4)       # [hg, wg, c, g, o]
        # wk64: tap i=2; taus {0,1,4,5} at partitions 0:64, {2,3} at 64:128
        wk64 = wkf[:, :, :, WR_COLS:WG_COLS]
        for tau in TAUS:
            nv, jlo = NV[tau], JLO[tau]
            if tau in K64LO:
                p0, cb = 0, K64LO[tau]
            else:
                p0, cb = 64, K64HI[tau]
            view = wk64[:, :, p0:p0 + 64, cb:cb + 4 * nv * 64].reshape(
                2, 16, 64, 4, nv, 64)
            for q in range(nv):
                j = jlo + q
                r = tau - j
                src = WT[r, 2].reshape(CIN, 2, 4, 16, 4, COUT)[:, :, j]
                view[:, :, :, :, q, :] = src.transpose(1, 2, 0, 3, 4)
        # [2, 16, 128, 4608] -> [16(hg*8+pi), 128, 9216]
        wk = _quant_w(wkf).reshape(2, 8, 2, 128, WG_COLS).transpose(
            0, 1, 3, 2, 4).reshape(16, 128, 2 * WG_COLS)
        wk = np.ascontiguousarray(wk)
        in_maps.append({"featA": fA, "wk": wk})
    return in_maps


def _gather(results, bias):
    out = np.zeros((B, COUT, HOUT, WOUT), dtype=np.float32)
    inv = 1.0 / float(WSCALE)
    for core, s in enumerate(STARTS):
        arr = np.asarray(results[core]["outS"]).astype(np.float32) * inv
        # [hg, g, b, wg, j, o] -> [b, o, hg, j, wg, g]
        arr = arr.reshape(2, 4, 32, 16, 4, 64).transpose(2, 5, 0, 4, 3, 1)
        arr = arr.reshape(32, 64, 8, 64)
        out[:, :, s:s + 8, 0:60] = arr[:, :, :, 0:60]
        out[:, :, s:s + 8, 60:62] = arr[:, :, :, 62:64]
    out += np.asarray(bias, dtype=np.float32).transpose(2, 0, 1)[None]
    return out


def _run(in_maps, trace=False, trace_cores=None):
    from concourse.bass_utils import run_bass_kernel_spmd
    nc = _get_nc()
    return run_bass_kernel_spmd(
        nc, in_maps, core_ids=list(range(NCORES)),
        trace=trace, trace_cores=trace_cores,
    )


def kernel(features, weights, bias):
    in_maps = _prep_inputs(features, weights)
    res = _run(in_maps)
    return _gather(res.results, bias)


# revision 9
# speedup vs baseline: 1.7232x; 1.0158x over previous
"""LocallyConnected2d kernel for 8 TRN2 NeuronCores (Bass/Tile).

Problem (hardcoded):
  features [32, 64, 64, 64] f32, weights [62, 62, 64, 64, 3, 3] f32,
  bias [62, 62, 64] f32 -> out [32, 64, 62, 62] f32
  out[b,o,h,w] = sum_{c,i,j} x[b,c,h+i,w+j] * W[h,w,o,c,i,j] + bias[h,w,o]

Strategy (v3 - fp8 weight stream, no featB, packed K=64 weights):
  - Shard over Hout: 8 cores x 8 output rows (bands [0,8,...,48,54], last two
    overlap; host takes canonical rows from each core).
  - Weights stream as fp8 e3m4 (x2 scale, /2 on host) = 1 B/el -> 18.9 MB/core
    with zero padding waste; activations stay bf16 (mixed-dtype matmul: only
    fp32 operands must be paired). PSUM accumulates fp32. rel err ~0.015.
  - fA layout [128=(c | c shifted w+1), w, t, b]; a [128,32] slice at (w,t) is
    an im2col patch: lower half = x(w), upper = x(w+1).
  - Per (hg=4-row half-band, wg=4-w group), outputs live in PSUM [128,256]:
    partitions=(4w x 32b) via col tile_position, free=(4j x 64 cout).
    * wr MMs (taps i in {0,1}): stationary fA[:, w0+g, hl+tau] K=128, moving
      N=nv*64, tau=j+r grouped. 24 MMs.
    * tap i=2 is K=64, tau-grouped, split across TWO psum groups because one
      accumulation group must not mix PE row-tiles (HW limitation, probed):
      - taus {0,1,4,5} (1536 cols): stationary fA[0:64, w0+g+2, hl+tau]
        (lower half), moving wk64[0:64, .], row tile 0, accumulate into main.
      - taus {2,3} (1536 cols): stationary fA[64:128, w0+g+1, hl+tau] (the
        w+1-shifted half gives the same x column), moving wk64[64:128, .],
        row tile 64, own psum tile psB with a row-64 K=1 zeroing matmul as
        group start. tau2 covers j0-2, tau3 j1-3 -> full coverage.
      Both partition halves of wk64 carry real data -> no padding bytes.
    * combine on DVE: copy psB -> S slice (bf16), tensor_add(S, ps, S).
  - DMA: fA w-chunk [0:8] FIRST on sync (655 KB -> compute starts early), the
    16 wk pair-transfers ([128,9216] fp8 = 1.18 MB) follow on sync; remaining
    fA chunks + outS dumps ride the scalar ring.
"""

import numpy as np
import ml_dtypes

BF16 = ml_dtypes.bfloat16
F8E3 = ml_dtypes.float8_e3m4
WSCALE = np.float32(2.0)

B, CIN, COUT = 32, 64, 64
H = W = 64
HOUT = WOUT = 62
NCORES = 8
STARTS = [0, 8, 16, 24, 32, 40, 48, 54]

# tau-group geometry: tau = t - hl in 0..5; valid out-rows j in [jlo, jhi]
TAUS = list(range(6))
JLO = [max(0, t - 2) for t in TAUS]
JHI = [min(3, t) for t in TAUS]
NV = [hi - lo + 1 for lo, hi in zip(JLO, JHI)]            # [1,2,3,3,2,1]
TBASE = [0]
for t in TAUS:
    TBASE.append(TBASE[-1] + 4 * NV[t] * 64)              # wr per-tau base col
WR_COLS = TBASE[-1]                                        # 3072
# K64 split: taus 0,1,4,5 -> lower half (row tile 0, main psum);
#            taus 2,3     -> upper half (row tile 64, psB)
K64_LOW_TAUS = [0, 1, 4, 5]
K64_HIGH_TAUS = [2, 3]
K64LO = {}
off = 0
for t in K64_LOW_TAUS:
    K64LO[t] = off
    off += 4 * NV[t] * 64
K64HI = {}
off = 0
for t in K64_HIGH_TAUS:
    K64HI[t] = off
    off += 4 * NV[t] * 64
K64_COLS = 1536                                            # both halves
WG_COLS = WR_COLS + K64_COLS                               # 4608

_STATE = {}


def _build_program():
    import concourse.tile as tile
    from concourse import bacc, mybir

    bf = mybir.dt.bfloat16
    f8 = mybir.dt.float8e3
    f32 = mybir.dt.float32
    ACT_COPY = mybir.ActivationFunctionType.Copy

    nc = bacc.Bacc(None, target_bir_lowering=False)
    featA = nc.dram_tensor("featA", [128, 64, 10, 32], bf, kind="ExternalInput")
    wk_d = nc.dram_tensor("wk", [16, 128, 2 * WG_COLS], f8, kind="ExternalInput")
    outS = nc.dram_tensor("outS", [2, 128, 4096], bf, kind="ExternalOutput")

    with tile.TileContext(nc) as tc:
        with tc.tile_pool(name="feat", bufs=1) as fpool, \
             tc.tile_pool(name="wk", bufs=4) as wkpool, \
             tc.tile_pool(name="st", bufs=2) as spool, \
             tc.tile_pool(name="ps", bufs=4, space="PSUM") as pspool, \
             tc.tile_pool(name="psb", bufs=4, space="PSUM") as psbpool:
            fA = fpool.tile([128, 64, 10, 32], bf)
            # first w-chunk ahead of the weight stream on sync; the rest on
            # scalar so they don't delay wk[0]
            nc.sync.dma_start(fA[:, 0:8], featA[:, 0:8])
            nc.scalar.dma_start(fA[:, 8:24], featA[:, 8:24])
            nc.scalar.dma_start(fA[:, 24:44], featA[:, 24:44])
            nc.scalar.dma_start(fA[:, 44:64], featA[:, 44:64])
            # zeros spanning all partitions: K=1 stationary rows for the
            # psum-clearing matmuls (row 0 for main, row 64 for psB)
            zb = fpool.tile([128, 256], bf)
            nc.gpsimd.memset(zb[:], 0.0)
            for hg in range(2):
                hl = 4 * hg
                S = spool.tile([128, 4096], bf)
                for pi in range(8):
                    wk = wkpool.tile([128, 2 * WG_COLS], f8)
                    if hg == 0 and pi == 0:
                        # split the first pair so wk[0] lands sooner and the
                        # first matmuls start earlier
                        nc.sync.dma_start(wk[:, 0:WG_COLS],
                                          wk_d[0][:, 0:WG_COLS])
                        nc.sync.dma_start(wk[:, WG_COLS:2 * WG_COLS],
                                          wk_d[0][:, WG_COLS:2 * WG_COLS])
                    else:
                        nc.sync.dma_start(wk[:], wk_d[hg * 8 + pi])
                    if pi == 4:
                        nc.scalar.dma_start(outS[hg][:, 0:2048],
                                            S[:, 0:2048])
                    for sub in range(2):
                        wg = 2 * pi + sub
                        w0 = min(4 * wg, 58)  # last group overlaps: w 58..61
                        wr = wk[:, sub * WG_COLS:sub * WG_COLS + WR_COLS]
                        wk64 = wk[:, sub * WG_COLS + WR_COLS:
                                  (sub + 1) * WG_COLS]

                        ps = pspool.tile([128, 256], f32)
                        psB = psbpool.tile([128, 256], f32)
                        # Zero PSUM off the PE: ACT copy-from-zeros for ps,
                        # DVE memset for psB. All matmuls then run flags=0:
                        # has_written (never cleared - no start=True) makes
                        # them accumulate onto the zeros where stale-set and
                        # overwrite the zeros where clear - correct either
                        # way, and the PE saves the K=1 zeroing matmuls.
                        nc.scalar.activation(ps[:, :], zb[:, :], ACT_COPY)
                        nc.vector.memset(psB[:, :], 0.0)
                        # taps i in {0,1}: K=128 dual-w stationaries
                        for tau in TAUS:
                            nv, jlo = NV[tau], JLO[tau]
                            for g in range(4):
                                off = TBASE[tau] + g * nv * 64
                                nc.tensor.matmul(
                                    ps[32 * g:32 * g + 32,
                                       64 * jlo:64 * (jlo + nv)],
                                    fA[:, w0 + g, hl + tau, :],
                                    wr[:, off:off + nv * 64],
                                    start=False, stop=False,
                                    skip_group_check=True,
                                    tile_position=(0, 32 * g),
                                )
                        # tap i=2, taus {0,1,4,5}: K=64 lower halves -> main
                        for ti, tau in enumerate(K64_LOW_TAUS):
                            nv, jlo = NV[tau], JLO[tau]
                            for g in range(4):
                                off = K64LO[tau] + g * nv * 64
                                nc.tensor.matmul(
                                    ps[32 * g:32 * g + 32,
                                       64 * jlo:64 * (jlo + nv)],
                                    fA[0:64, w0 + g + 2, hl + tau, :],
                                    wk64[0:64, off:off + nv * 64],
                                    start=False,
                                    stop=(ti == 3 and g == 3),
                                    skip_group_check=True,
                                    tile_position=(0, 32 * g),
                                )
                        # tap i=2, taus {2,3}: K=64 upper halves -> psB
                        for ti, tau in enumerate(K64_HIGH_TAUS):
                            nv, jlo = NV[tau], JLO[tau]
                            for g in range(4):
                                off = K64HI[tau] + g * nv * 64
                                nc.tensor.matmul(
                                    psB[32 * g:32 * g + 32,
                                        64 * jlo:64 * (jlo + nv)],
                                    fA[64:128, w0 + g + 1, hl + tau, :],
                                    wk64[64:128, off:off + nv * 64],
                                    start=False,
                                    stop=(ti == 1 and g == 3),
                                    skip_group_check=True,
                                    tile_position=(64, 32 * g),
                                )
                        sl = S[:, 256 * wg:256 * wg + 256]
                        nc.scalar.activation(sl, psB[:, :], ACT_COPY)
                        nc.vector.tensor_add(sl, ps[:], sl)
                nc.scalar.dma_start(outS[hg][:, 2048:4096], S[:, 2048:4096])
    nc.compile()
    return nc


def _get_nc():
    if "nc" not in _STATE:
        _STATE["nc"] = _build_program()
    return _STATE["nc"]


def _quant_w(a):
    return np.clip(a * WSCALE, -15.0, 15.0).astype(F8E3)


def _prep_inputs(features, weights):
    """Build the 8 per-core input dicts (device layouts)."""
    x = np.asarray(features, dtype=np.float32)
    Wt = np.asarray(weights, dtype=np.float32)

    # w-slot -> real w: last group overlaps (w 58..61)
    widx = list(range(60)) + [58, 59, 60, 61]

    in_maps = []
    for s in STARTS:
        xt = x[:, :, s:s + 10, :].transpose(1, 3, 2, 0)    # [c, w, t, b]
        fA = np.zeros((128, 64, 10, 32), dtype=BF16)
        fA[:64] = xt
        fA[64:, 0:63] = xt[:, 1:]                          # w+1 shift

        Wb = Wt[s:s + 8]                                   # [8, 62, o, c, 3, 3]
        Wsel = Wb[:, widx]                                 # [8, 64slots, o, c, 3, 3]
        WT = Wsel.transpose(4, 5, 3, 0, 1, 2)              # [r, i, c, 8h, 64w, o]

        wkf = np.zeros((2, 16, 128, WG_COLS), dtype=np.float32)
        # wr: taps (r, i=d); cols per (tau, g): q -> j=jlo+q, r=tau-j
        wr = wkf[:, :, :, 0:WR_COLS]
        for tau in TAUS:
            nv, jlo = NV[tau], JLO[tau]
            view = wr[:, :, :, TBASE[tau]:TBASE[tau + 1]].reshape(
                2, 16, 128, 4, nv, 64)
            for q in range(nv):
                j = jlo + q
                r = tau - j
                for d in range(2):
                    src = WT[r, d].reshape(CIN, 2, 4, 16, 4, COUT)[:, :, j]
                    view[:, :, d * 64:(d + 1) * 64, :, q, :] = \
                        src.transpose(1, 2, 0, 3, 4)       # [hg, wg, c, g, o]
        # wk64: tap i=2; taus {0,1,4,5} at partitions 0:64, {2,3} at 64:128
        wk64 = wkf[:, :, :, WR_COLS:WG_COLS]
        for tau in TAUS:
            nv, jlo = NV[tau], JLO[tau]
            if tau in K64LO:
                p0, cb = 0, K64LO[tau]
            else:
                p0, cb = 64, K64HI[tau]
            view = wk64[:, :, p0:p0 + 64, cb:cb + 4 * nv * 64].reshape(
                2, 16, 64, 4, nv, 64)
            for q in range(nv):
                j = jlo + q
                r = tau - j
                src = WT[r, 2].reshape(CIN, 2, 4, 16, 4, COUT)[:, :, j]
                view[:, :, :, :, q, :] = src.transpose(1, 2, 0, 3, 4)
        # [2, 16, 128, 4608] -> [16(hg*8+pi), 128, 9216]
        wk = _quant_w(wkf).reshape(2, 8, 2, 128, WG_COLS).transpose(
            0, 1, 3, 2, 4).reshape(16, 128, 2 * WG_COLS)
        wk = np.ascontiguousarray(wk)
        in_maps.append({"featA": fA, "wk": wk})
    return in_maps


def _gather(results, bias):
    out = np.zeros((B, COUT, HOUT, WOUT), dtype=np.float32)
    inv = 1.0 / float(WSCALE)
    for core, s in enumerate(STARTS):
        arr = np.asarray(results[core]["outS"]).astype(np.float32) * inv
        # [hg, g, b, wg, j, o] -> [b, o, hg, j, wg, g]
        arr = arr.reshape(2, 4, 32, 16, 4, 64).transpose(2, 5, 0, 4, 3, 1)
        arr = arr.reshape(32, 64, 8, 64)
        out[:, :, s:s + 8, 0:60] = arr[:, :, :, 0:60]
        out[:, :, s:s + 8, 60:62] = arr[:, :, :, 62:64]
    out += np.asarray(bias, dtype=np.float32).transpose(2, 0, 1)[None]
    return out


def _run(in_maps, trace=False, trace_cores=None):
    from concourse.bass_utils import run_bass_kernel_spmd
    nc = _get_nc()
    return run_bass_kernel_spmd(
        nc, in_maps, core_ids=list(range(NCORES)),
        trace=trace, trace_cores=trace_cores,
    )


def kernel(features, weights, bias):
    in_maps = _prep_inputs(features, weights)
    res = _run(in_maps)
    return _gather(res.results, bias)
